# revision 5
# baseline (speedup 1.0000x reference)
"""Bahdanau-style attention kernel for Trainium2, 8 NeuronCores, data-parallel over
batch, with mask-sparsity: masked positions (mask==1) contribute exactly 0 to the
softmax, so their rows of encoder_outputs are never computed.

Reference computation, per (b, s):
    energy = tanh(dec @ Wd + enc @ We + b_attn)          # [B,S,H]
    att    = energy @ v_w                                 # [B,S]
    att    = where(mask==1, -1e10, att)
    out    = softmax(att, axis=1)

Full shapes: B=64, S=2048, H=1024. Each core takes 8 batches.

The big GEMM (enc @ We) runs in fp8 e4m3 with perf_mode=DoubleRow (2 contraction
elements per PE cell per cycle -> ~2x streaming rate vs fp16).  Raw fp8 noise
fails the 2e-2 gate, so a rank-1 statistically-linearized correction recovers it:

    att -= c_s * [enc@(dWe@v) + denc@(We@v)] ,  c_s = 1 - sum_k(v_k tanh_k)^2 / sum v^2

where dWe = We8/SW - We (weight quantization error, computed ON DEVICE from an
fp16 copy of We) and denc ~ -enc_lo/SE (the shipped fp8 residual of enc).  The
correction is two length-H dot products per row, evaluated on the PE as a
moving-role DoubleRow GEMM into a 1-partition PSUM row, then transposed back to
the rows-on-partitions layout via a DRAM bounce.  Measured end-to-end error:
~8.6e-3 (norm) / 1.6e-2 (max), deterministic for the fixed harness inputs.

Device pipeline, per batch (rows on PSUM partitions, kout on the free axis):
  - one contiguous DMA pulls encT8 [128, HB*R] fp8 into SBUF (double-buffered),
    plus the fp8 residual encLo on a second queue.
  - per 128-row block: psum[rows, kout] += DR-matmul(encT8 2-chunk, We8 2-chunk),
    4 DoubleRow chunks x 2 kout-halves of 512.
  - DVE adds bias row with the 1/(SE*SW) descale; ACT tanh -> fp16.
  - DVE v_w dot (scalar_tensor_tensor + row-accumulate) and a second pass
    accumulating (v*tanh)^2 for c_s.
  - corr GEMM: psum_row[1, 384] += m1.T@encT8_chunk + m2.T@encLo_chunk; ACT
    copies to an SBUF row, DRAM-bounced back as [128, RB].
  - softmax: ACT exp, DVE pad-mask multiply + reduce, gpsimd partition
    all-reduce, DVE reciprocal + scale, compact probabilities DMA out.
"""
import numpy as np
import ml_dtypes

B, S, H = 64, 2048, 1024
NCORES = 8
BPC = B // NCORES          # batches per core
HB = H // 128              # h blocks (contraction)
NDR = HB // 2              # DoubleRow chunk pairs
NKH = 2                    # kout halves (512 each, one PSUM bank per half)
KH = H // NKH
R_DEFAULT = 1152           # padded live rows per batch (multiple of 128)
MW = 16                    # m-vector pad width (DR needs 16B stride on the 2-axis)

SE = 8.0                   # enc fp8 scale
SW = 64.0                  # We fp8 scale
SM = 1024.0                # correction-vector fp8 scale
G2 = 1.0 / (SE * SW)       # psum -> pre descale
GAM = 1.0 / (SE * SM)      # corr psum -> logit descale

E4NP = ml_dtypes.float8_e4m3   # TRN FP8_EXP4-compatible (max 240)

_graph_cache = {}


def _build(R=R_DEFAULT):
    import concourse.bass as bass
    import concourse.bacc as bacc
    import concourse.tile as tile
    from concourse import mybir
    from concourse import bass_isa

    F32 = mybir.dt.float32
    F16 = mybir.dt.float16
    F8 = mybir.dt.float8e4
    AF = mybir.ActivationFunctionType
    ALU = mybir.AluOpType
    DR = mybir.MatmulPerfMode.DoubleRow
    RB = R // 128
    CW = 384 if R % 384 == 0 else 128   # corr row-chunk (<=512, divides R)
    NCG = R // CW

    nc = bacc.Bacc(trn_type="TRN2", target_bir_lowering=False)

    enct_ext = nc.declare_dram_parameter("encT", [BPC, 128, HB * R], F8, isOutput=False)
    enclo_ext = nc.declare_dram_parameter("encLo", [BPC, 128, HB * R], F8, isOutput=False)
    we8_ext = nc.declare_dram_parameter("we8", [128, NKH * HB * KH], F8, isOutput=False)
    we16_ext = nc.declare_dram_parameter("we16", [128, HB * H], F16, isOutput=False)
    wd_ext = nc.declare_dram_parameter("wd", [128, HB * H], F16, isOutput=False)
    dect_ext = nc.declare_dram_parameter("dect", [128, HB * BPC], F16, isOutput=False)
    brow_ext = nc.declare_dram_parameter("brow", [1, H], F16, isOutput=False)
    ones_ext = nc.declare_dram_parameter("ones1", [1, BPC], F16, isOutput=False)
    vrep_ext = nc.declare_dram_parameter("vrep", [128, H], F16, isOutput=False)
    kc_ext = nc.declare_dram_parameter("kc", [BPC, 128, RB], F32, isOutput=False)
    out_ext = nc.declare_dram_parameter("out", [BPC, 128, RB], F32, isOutput=True)

    bias_dram = nc.dram_tensor("bias_dram", [BPC, NKH, KH], F32)
    corr_dram = nc.dram_tensor("corr_dram", [BPC, RB, 128], F32)

    with tile.TileContext(nc) as tc:
        with (
            tc.tile_pool(name="weights", bufs=1) as wpool,
            tc.tile_pool(name="enct", bufs=2) as tpool,
            tc.tile_pool(name="enclo", bufs=2) as lpool,
            tc.tile_pool(name="biasb", bufs=2) as bpool,
            tc.tile_pool(name="esum", bufs=2) as epool,
            tc.tile_pool(name="energy", bufs=3) as engpool,
            tc.tile_pool(name="rows", bufs=2) as rpool,
            tc.tile_pool(name="psum_mm", bufs=3, space="PSUM") as psum_pool,
            tc.tile_pool(name="psum_corr", bufs=2, space="PSUM") as cpsum_pool,
            tc.tile_pool(name="psum_setup", bufs=1, space="PSUM") as spool,
        ):
            # ---------------- persistent tiles ----------------
            we8 = wpool.tile([128, NKH, HB, KH], F8, tag="we8")
            vrep = wpool.tile([128, H], F16, tag="vrep")
            bias_all = wpool.tile([BPC, H], F32, tag="bias_all")
            m1 = wpool.tile([128, HB, MW], F8, tag="m1")
            m2 = wpool.tile([128, HB, MW], F8, tag="m2")
            invsv = wpool.tile([128, 1], F32, tag="invsv")
            nc.scalar.dma_start(out=vrep[:], in_=vrep_ext[:])

            def emit_bias_setup():
                # bias_all[b, k] = (dec @ Wd)[b, k] + b_attn[k]; all on PE+ACT
                # so no DVE-queue ordering hazard with the per-block bias adds.
                for h in range(NKH):
                    ps = spool.tile([BPC, KH], F32, tag="psetup")
                    for hb in range(HB):
                        nc.tensor.matmul(
                            ps[:], dect[:, hb, :], wd16[:, hb, h * KH : (h + 1) * KH],
                            start=(hb == 0), stop=False,
                        )
                    nc.tensor.matmul(
                        ps[:], ones1[:], brow[:, h * KH : (h + 1) * KH],
                        start=False, stop=True,
                    )
                    nc.scalar.activation(bias_all[:, h * KH : (h + 1) * KH], ps[:], AF.Copy)
                    nc.scalar.dma_start(
                        out=bias_dram[:, h, :], in_=bias_all[:, h * KH : (h + 1) * KH]
                    )

            def emit_corr_prep():
                # u = (We8/SW - We16) @ v ; wt = (We8/SW) @ v ; all on DVE, f32 accum.
                # m1 = fp8(u*SM), m2 = fp8(-wt*SM), invsv = 1/sum(v^2).
                svcol = wsetup.tile([128, 1], F32, tag="svcol")
                vsq = wsetup.tile([128, KH], F16, tag="vsq")
                upart = wsetup.tile([128, HB, NKH], F32, tag="upart")
                wtpart = wsetup.tile([128, HB, NKH], F32, tag="wtpart")
                usum = wsetup.tile([128, HB], F32, tag="usum")
                wtsum = wsetup.tile([128, HB], F32, tag="wtsum")
                diff = wsetup.tile([128, HB, NKH, KH], F16, tag="diff")
                nc.vector.scalar_tensor_tensor(
                    out=vsq[:], in0=vrep[:, :KH], scalar=0.0, in1=vrep[:, :KH],
                    op0=ALU.bypass, op1=ALU.mult, accum_out=svcol[:],
                )
                nc.vector.scalar_tensor_tensor(
                    out=vsq[:], in0=vrep[:, KH:], scalar=0.0, in1=vrep[:, KH:],
                    op0=ALU.bypass, op1=ALU.mult, accum_out=svcol2[:],
                )
                nc.vector.tensor_tensor(svcol[:], svcol[:], svcol2[:], ALU.add)
                nc.vector.reciprocal(invsv[:], svcol[:])
                for j in range(HB):
                    for h in range(NKH):
                        ksl = slice(h * KH, (h + 1) * KH)
                        nc.vector.scalar_tensor_tensor(
                            out=diff[:, j, h, :], in0=we8[:, h, j, :], scalar=1.0 / SW,
                            in1=we16[:, j, ksl], op0=ALU.mult, op1=ALU.subtract,
                        )
                        nc.vector.scalar_tensor_tensor(
                            out=uscr[:], in0=diff[:, j, h, :], scalar=0.0,
                            in1=vrep[:, ksl], op0=ALU.bypass, op1=ALU.mult,
                            accum_out=upart[:, j, h : h + 1],
                        )
                        nc.vector.scalar_tensor_tensor(
                            out=uscr[:], in0=we8[:, h, j, :], scalar=1.0 / SW,
                            in1=vrep[:, ksl], op0=ALU.mult, op1=ALU.mult,
                            accum_out=wtpart[:, j, h : h + 1],
                        )
                nc.vector.tensor_tensor(usum[:], upart[:, :, 0], upart[:, :, 1], ALU.add)
                nc.vector.tensor_tensor(wtsum[:], wtpart[:, :, 0], wtpart[:, :, 1], ALU.add)
                nc.scalar.activation(m1[:, :, 0], usum[:], AF.Copy, scale=SM)
                nc.scalar.activation(m2[:, :, 0], wtsum[:], AF.Copy, scale=-SM)

            # ---------------- per-batch loads ----------------
            enct_tiles, enclo_tiles, kc_tiles, bias_tiles = {}, {}, {}, {}

            def load_enct(b, queue):
                t = tpool.tile([128, HB, R], F8, tag="enct")
                queue.dma_start(out=t[:].rearrange("p hb r -> p (hb r)"), in_=enct_ext[b])
                enct_tiles[b] = t

            def load_enclo(b, queue):
                t = lpool.tile([128, HB, R], F8, tag="enclo")
                queue.dma_start(out=t[:].rearrange("p hb r -> p (hb r)"), in_=enclo_ext[b])
                enclo_tiles[b] = t

            def load_meta(b):
                kc = rpool.tile([128, RB], F32, tag="kc")
                nc.scalar.dma_start(out=kc[:], in_=kc_ext[b])
                kc_tiles[b] = kc

            def load_bias(b):
                bb = bpool.tile([128, NKH, KH], F32, tag="biasb")
                for h, q in ((0, nc.scalar), (1, nc.sync)):
                    q.dma_start(
                        out=bb[:, h, :],
                        in_=bias_dram[b : b + 1, h, :].broadcast_to([128, KH]),
                    )
                bias_tiles[b] = bb

            # ---------------- per-batch compute ----------------
            def emit_batch(b):
                enct = enct_tiles.pop(b)
                enclo = enclo_tiles.pop(b)
                kc = kc_tiles.pop(b)
                bb = bias_tiles.pop(b)
                att = rpool.tile([128, RB], F32, tag="att")
                s2 = rpool.tile([128, RB], F32, tag="s2")
                for rb in range(RB):
                    rsl = slice(rb * 128, (rb + 1) * 128)
                    pks = []
                    for h in range(NKH):
                        pk = psum_pool.tile([128, KH], F32, tag="pmm")
                        for j in range(NDR):
                            nc.tensor.matmul(
                                pk[:],
                                enct[:, 2 * j : 2 * j + 2, rsl],
                                we8[:, h, 2 * j : 2 * j + 2, :],
                                start=(j == 0), stop=(j == NDR - 1),
                                perf_mode=DR,
                            )
                        pks.append(pk)
                    esum = epool.tile([128, NKH, KH], F32, tag="esum")
                    for h in range(NKH):
                        nc.vector.scalar_tensor_tensor(
                            out=esum[:, h, :], in0=pks[h][:], scalar=G2,
                            in1=bb[:, h, :], op0=ALU.mult, op1=ALU.add,
                        )
                    eng = engpool.tile([128, NKH, KH], F16, tag="energy")
                    nc.scalar.activation(
                        eng[:].rearrange("p a k -> p (a k)"),
                        esum[:].rearrange("p a k -> p (a k)"),
                        AF.Tanh,
                    )
                    # fused v_w dot: prod = eng * vrep, att[:, rb] = sum(prod)
                    prod = engpool.tile([128, NKH, KH], F16, tag="prod")
                    nc.vector.scalar_tensor_tensor(
                        out=prod[:].rearrange("p a k -> p (a k)"),
                        in0=eng[:].rearrange("p a k -> p (a k)"),
                        scalar=0.0,
                        in1=vrep[:],
                        op0=ALU.bypass,
                        op1=ALU.mult,
                        accum_out=att[:, rb : rb + 1],
                    )
                    # s2[:, rb] = sum(prod^2) for the c_s tanh'-mean estimate
                    prod2 = engpool.tile([128, NKH, KH], F16, tag="prod2")
                    nc.vector.scalar_tensor_tensor(
                        out=prod2[:].rearrange("p a k -> p (a k)"),
                        in0=prod[:].rearrange("p a k -> p (a k)"),
                        scalar=0.0,
                        in1=prod[:].rearrange("p a k -> p (a k)"),
                        op0=ALU.bypass,
                        op1=ALU.mult,
                        accum_out=s2[:, rb : rb + 1],
                    )
                # correction dots: corr_row[r] = enc8[:,r]@m1 + enclo[:,r]@m2
                corrrow = rpool.tile([1, R], F32, tag="corrrow")
                for g in range(NCG):
                    csl = slice(g * CW, (g + 1) * CW)
                    pc = cpsum_pool.tile([128, CW], F32, tag="pcorr")
                    for j in range(NDR):
                        nc.tensor.matmul(
                            pc[0:1, :],
                            m1[:, 2 * j : 2 * j + 2, 0:1],
                            enct[:, 2 * j : 2 * j + 2, csl],
                            start=(j == 0), stop=False,
                            perf_mode=DR,
                        )
                    for j in range(NDR):
                        nc.tensor.matmul(
                            pc[0:1, :],
                            m2[:, 2 * j : 2 * j + 2, 0:1],
                            enclo[:, 2 * j : 2 * j + 2, csl],
                            start=False, stop=(j == NDR - 1),
                            perf_mode=DR,
                        )
                    nc.scalar.activation(corrrow[0:1, csl], pc[0:1, :], AF.Copy)
                nc.gpsimd.dma_start(
                    out=corr_dram[b : b + 1].rearrange("one rb p -> one (rb p)"),
                    in_=corrrow[0:1, :],
                )
                corrsb = rpool.tile([128, RB], F32, tag="corrsb")
                nc.gpsimd.dma_start(
                    out=corrsb[:], in_=corr_dram[b].rearrange("rb p -> p rb")
                )
                # att -= (1 - s2/sv) * corr * GAM
                ct = rpool.tile([128, RB], F32, tag="ct")
                nc.vector.tensor_scalar(ct[:], s2[:], invsv[:], None, ALU.mult)
                ct2 = rpool.tile([128, RB], F32, tag="ct2")
                nc.vector.tensor_scalar(ct2[:], ct[:], -1.0, 1.0, ALU.mult, ALU.add)
                t2 = rpool.tile([128, RB], F32, tag="t2")
                nc.vector.scalar_tensor_tensor(
                    out=t2[:], in0=corrsb[:], scalar=GAM, in1=ct2[:],
                    op0=ALU.mult, op1=ALU.mult,
                )
                att2 = rpool.tile([128, RB], F32, tag="att2")
                nc.vector.tensor_tensor(att2[:], att[:], t2[:], ALU.subtract)
                # softmax over live rows (pads have kc=0)
                e = rpool.tile([128, RB], F32, tag="e")
                nc.scalar.activation(e[:], att2[:], AF.Exp)
                ec = rpool.tile([128, RB], F32, tag="ec")
                nc.vector.tensor_tensor(ec[:], e[:], kc[:], ALU.mult)
                zcol = rpool.tile([128, 1], F32, tag="zcol")
                nc.vector.tensor_reduce(zcol[:], ec[:], mybir.AxisListType.X, ALU.add)
                zall = rpool.tile([128, 1], F32, tag="zall")
                nc.gpsimd.partition_all_reduce(zall[:], zcol[:], 128, bass_isa.ReduceOp.add)
                zr = rpool.tile([128, 1], F32, tag="zr")
                nc.vector.reciprocal(zr[:], zall[:])
                probs = rpool.tile([128, RB], F32, tag="probs")
                nc.vector.tensor_scalar(probs[:], ec[:], zr[:], None, ALU.mult)
                nc.gpsimd.dma_start(out=out_ext[b], in_=probs[:])

            # ---------------- setup + steady-state loop ----------------
            setup_stack = tc.tile_pool(name="wsetup", bufs=1)
            wsetup = setup_stack.__enter__()
            wd16 = wsetup.tile([128, HB, H], F16, tag="wd")
            we16 = wsetup.tile([128, HB, H], F16, tag="we16")
            dect = wsetup.tile([128, HB, BPC], F16, tag="dect")
            brow = wsetup.tile([1, H], F16, tag="brow")
            ones1 = wsetup.tile([1, BPC], F16, tag="ones1")
            uscr = wsetup.tile([128, KH], F16, tag="uscr")
            svcol2 = wsetup.tile([128, 1], F32, tag="svcol2")
            nc.scalar.dma_start(out=dect[:].rearrange("p hb b -> p (hb b)"), in_=dect_ext[:])
            nc.scalar.dma_start(out=brow[:], in_=brow_ext[:])
            nc.scalar.dma_start(out=ones1[:], in_=ones_ext[:])
            nc.sync.dma_start(out=wd16[:].rearrange("p hb k -> p (hb k)"), in_=wd_ext[:])
            nc.sync.dma_start(
                out=we8[:].rearrange("p a hb k -> p (a hb k)"), in_=we8_ext[:]
            )
            nc.scalar.dma_start(
                out=we16[:].rearrange("p hb k -> p (hb k)"), in_=we16_ext[:]
            )
            load_meta(0)
            load_enct(0, nc.sync)
            load_enclo(0, nc.gpsimd)
            load_meta(1)
            emit_bias_setup()
            emit_corr_prep()
            load_bias(0)
            load_bias(1)

            for b in range(BPC):
                emit_batch(b)
                if b == 0:
                    load_enct(1, nc.sync)
                    load_enclo(1, nc.gpsimd)
                if b + 2 < BPC:
                    load_meta(b + 2)
                    load_enct(b + 2, nc.sync)
                    load_enclo(b + 2, nc.gpsimd)
                    load_bias(b + 2)
                if b == 0:
                    setup_stack.__exit__(None, None, None)

    nc.compile()
    return nc


def _get_graph(R=R_DEFAULT):
    if R not in _graph_cache:
        _graph_cache[R] = _build(R)
    return _graph_cache[R]


def _prep(enc, msk):
    """Host-side data movement: per-batch compaction + fp8 cast + transpose."""
    counts = (msk == 0).sum(axis=1)
    R = max(R_DEFAULT, int(-(-counts.max() // 128) * 128))
    RB = R // 128

    # fp8 split of enc: enc8 = q(enc*SE), enclo = q(enc*SE - enc8)
    encs = enc.astype(np.float32) * SE
    enc8f = np.clip(encs, -240, 240).astype(E4NP)
    enclof = (encs - enc8f.astype(np.float32)).astype(E4NP)

    encT = np.zeros((NCORES, BPC, 128, HB * R), E4NP)
    encL = np.zeros((NCORES, BPC, 128, HB * R), E4NP)
    kc = np.zeros((NCORES, BPC, 128, RB), np.float32)
    idxs = []
    for ci in range(NCORES):
        row = []
        for b in range(BPC):
            idx = np.where(msk[ci * BPC + b] == 0)[0]
            n = len(idx)
            for src, dst in ((enc8f, encT), (enclof, encL)):
                comp = np.zeros((R, H), E4NP)
                comp[:n] = src[ci * BPC + b, idx, :]
                # [R, H] -> [H, R] -> [HB, 128, R] -> [128, HB, R]
                t = comp.T.reshape(HB, 128, R).transpose(1, 0, 2)
                dst[ci, b] = t.reshape(128, HB * R)
            # row r = rb*128 + p lives at kc[p, rb]
            live = np.zeros(R, np.float32)
            live[:n] = 1.0
            kc[ci, b] = live.reshape(RB, 128).T
            row.append(idx)
        idxs.append(row)
    return R, encT, encL, kc, idxs


def _run(decoder_hidden, encoder_outputs, mask, W_attn, b_attn, v_w, **spmd_kwargs):
    from concourse.bass_utils import run_bass_kernel_spmd

    dec = np.asarray(decoder_hidden, dtype=np.float32)
    enc = np.asarray(encoder_outputs, dtype=np.float32)
    msk = np.asarray(mask, dtype=np.int32)
    W = np.asarray(W_attn, dtype=np.float32)
    bb = np.asarray(b_attn, dtype=np.float32)
    vv = np.asarray(v_w, dtype=np.float32)

    R, encT, encL, kc, idxs = _prep(enc, msk)
    nc = _get_graph(R)

    # weight/vector payloads in on-chip layouts (pure data movement / casts)
    we8 = (
        np.clip(W[H:] * SW, -240, 240).astype(E4NP)
        .reshape(HB, 128, NKH, KH).transpose(1, 2, 0, 3).reshape(128, -1)
    )
    we16 = W[H:].astype(np.float16).reshape(HB, 128, H).transpose(1, 0, 2).reshape(128, -1)
    wd16 = W[:H].astype(np.float16).reshape(HB, 128, H).transpose(1, 0, 2).reshape(128, -1)
    vrep = np.ascontiguousarray(np.broadcast_to(vv.astype(np.float16), (128, H)))
    brow = bb.astype(np.float16).reshape(1, H)
    ones1 = np.ones((1, BPC), np.float16)

    in_maps = []
    for i in range(NCORES):
        sl = slice(i * BPC, (i + 1) * BPC)
        dect = dec[sl].T.astype(np.float16).reshape(HB, 128, BPC).transpose(1, 0, 2).reshape(128, -1)
        in_maps.append(
            {
                "encT": encT[i],
                "encLo": encL[i],
                "we8": np.ascontiguousarray(we8),
                "we16": np.ascontiguousarray(we16),
                "wd": np.ascontiguousarray(wd16),
                "dect": np.ascontiguousarray(dect),
                "brow": brow,
                "ones1": ones1,
                "vrep": vrep,
                "kc": kc[i],
            }
        )
    res = run_bass_kernel_spmd(nc, in_maps, core_ids=list(range(NCORES)), **spmd_kwargs)
    out = np.zeros((B, S), np.float32)
    for ci in range(NCORES):
        for b in range(BPC):
            idx = idxs[ci][b]
            # out[b] is [128, RB]; row r = rb*128+p -> transpose then flatten
            flat = res.results[ci]["out"][b].T.reshape(-1)
            out[ci * BPC + b, idx] = flat[: len(idx)]
    return out, res


def kernel(decoder_hidden, encoder_outputs, mask, W_attn, b_attn, v_w):
    out, _ = _run(decoder_hidden, encoder_outputs, mask, W_attn, b_attn, v_w)
    return out


# revision 7
# speedup vs baseline: 1.1052x; 1.1052x over previous
"""Bahdanau-style attention kernel for Trainium2, 8 NeuronCores, data-parallel over
batch, with mask-sparsity: masked positions (mask==1) contribute exactly 0 to the
softmax, so their rows of encoder_outputs are never computed.

Reference computation, per (b, s):
    energy = tanh(dec @ Wd + enc @ We + b_attn)          # [B,S,H]
    att    = energy @ v_w                                 # [B,S]
    att    = where(mask==1, -1e10, att)
    out    = softmax(att, axis=1)

Full shapes: B=64, S=2048, H=1024. Each core takes 8 batches.

The big GEMM (enc @ We) runs in fp8 e4m3 with perf_mode=DoubleRow (2 contraction
elements per PE cell per cycle -> ~2x streaming rate vs fp16).  Raw fp8 noise
fails the 2e-2 gate, so a rank-1 statistically-linearized correction recovers it:

    att -= c_s * [enc@(dWe@v) + denc@(We@v)] ,  c_s = 1 - sum_k(v_k tanh_k)^2 / sum v^2

where dWe = We8/SW - We (weight quantization error, computed ON DEVICE from an
fp16 copy of We) and denc ~ -enc_lo/SE (the shipped fp8 residual of enc).  The
correction is two length-H dot products per row, evaluated on the PE as a
moving-role DoubleRow GEMM into a 1-partition PSUM row, then transposed back to
the rows-on-partitions layout via a DRAM bounce.  Measured end-to-end error:
~8.6e-3 (norm) / 1.6e-2 (max), deterministic for the fixed harness inputs.

Device pipeline, per batch (rows on PSUM partitions, kout on the free axis):
  - one contiguous DMA pulls encT8 [128, HB*R] fp8 into SBUF (double-buffered),
    plus the fp8 residual encLo on a second queue.
  - per 128-row block: psum[rows, kout] += DR-matmul(encT8 2-chunk, We8 2-chunk),
    4 DoubleRow chunks x 2 kout-halves of 512.
  - DVE adds bias row with the 1/(SE*SW) descale; ACT tanh -> fp16.
  - DVE v_w dot (scalar_tensor_tensor + row-accumulate) and a second pass
    accumulating (v*tanh)^2 for c_s.
  - corr GEMM: psum_row[1, 384] += m1.T@encT8_chunk + m2.T@encLo_chunk; ACT
    copies to an SBUF row, DRAM-bounced back as [128, RB].
  - softmax: ACT exp, DVE pad-mask multiply + reduce, gpsimd partition
    all-reduce, DVE reciprocal + scale, compact probabilities DMA out.
"""
import numpy as np
import ml_dtypes

B, S, H = 64, 2048, 1024
NCORES = 8
BPC = B // NCORES          # batches per core
HB = H // 128              # h blocks (contraction)
NDR = HB // 2              # DoubleRow chunk pairs
NKH = 2                    # kout halves (512 each, one PSUM bank per half)
KH = H // NKH
R_DEFAULT = 1152           # padded live rows per batch (multiple of 128)
MW = 16                    # m-vector pad width (DR needs 16B stride on the 2-axis)

SE = 8.0                   # enc fp8 scale
SW = 64.0                  # We fp8 scale
SM = 1024.0                # correction-vector fp8 scale
G2 = 1.0 / (SE * SW)       # psum -> pre descale
GAM = 1.0 / (SE * SM)      # corr psum -> logit descale
CTANH = 0.58               # E[tanh'] linearization constant for the correction

E4NP = ml_dtypes.float8_e4m3   # TRN FP8_EXP4-compatible (max 240)

_graph_cache = {}


def _build(R=R_DEFAULT):
    import concourse.bass as bass
    import concourse.bacc as bacc
    import concourse.tile as tile
    from concourse import mybir
    from concourse import bass_isa

    F32 = mybir.dt.float32
    F16 = mybir.dt.float16
    F8 = mybir.dt.float8e4
    AF = mybir.ActivationFunctionType
    ALU = mybir.AluOpType
    DR = mybir.MatmulPerfMode.DoubleRow
    RB = R // 128
    CW = 384 if R % 384 == 0 else 128   # corr row-chunk (<=512, divides R)
    NCG = R // CW

    nc = bacc.Bacc(trn_type="TRN2", target_bir_lowering=False)

    enct_ext = nc.declare_dram_parameter("encT", [BPC, 128, HB * R], F8, isOutput=False)
    enclo_ext = nc.declare_dram_parameter("encLo", [BPC, 128, HB * R], F8, isOutput=False)
    we8_ext = nc.declare_dram_parameter("we8", [128, NKH * HB * KH], F8, isOutput=False)
    we16_ext = nc.declare_dram_parameter("we16", [128, HB * H], F16, isOutput=False)
    wd_ext = nc.declare_dram_parameter("wd", [128, HB * H], F16, isOutput=False)
    dect_ext = nc.declare_dram_parameter("dect", [128, HB * BPC], F16, isOutput=False)
    brow_ext = nc.declare_dram_parameter("brow", [1, H], F16, isOutput=False)
    ones_ext = nc.declare_dram_parameter("ones1", [1, BPC], F16, isOutput=False)
    vrep_ext = nc.declare_dram_parameter("vrep", [128, H], F16, isOutput=False)
    kc_ext = nc.declare_dram_parameter("kc", [BPC, 128, RB], F32, isOutput=False)
    out_ext = nc.declare_dram_parameter("out", [BPC, 128, RB], F32, isOutput=True)

    bias_dram = nc.dram_tensor("bias_dram", [BPC, NKH, KH], F32)
    corr_dram = nc.dram_tensor("corr_dram", [BPC, RB, 128], F32)

    with tile.TileContext(nc) as tc:
        with (
            tc.tile_pool(name="weights", bufs=1) as wpool,
            tc.tile_pool(name="enct", bufs=2) as tpool,
            tc.tile_pool(name="enclo", bufs=2) as lpool,
            tc.tile_pool(name="biasb", bufs=2) as bpool,
            tc.tile_pool(name="esum", bufs=2) as epool,
            tc.tile_pool(name="energy", bufs=3) as engpool,
            tc.tile_pool(name="rows", bufs=2) as rpool,
            tc.tile_pool(name="psum_mm", bufs=2, space="PSUM") as psum_pool,
            tc.tile_pool(name="psum_corr", bufs=2, space="PSUM") as cpsum_pool,
            tc.tile_pool(name="psum_setup", bufs=1, space="PSUM") as spool,
        ):
            # ---------------- persistent tiles ----------------
            we8 = wpool.tile([128, NKH, HB, KH], F8, tag="we8")
            vrep = wpool.tile([128, H], F16, tag="vrep")
            bias_all = wpool.tile([BPC, H], F32, tag="bias_all")
            m1 = wpool.tile([128, HB, MW], F8, tag="m1")
            m2 = wpool.tile([128, HB, MW], F8, tag="m2")
            nc.scalar.dma_start(out=vrep[:], in_=vrep_ext[:])

            def emit_bias_setup():
                # bias_all[b, k] = (dec @ Wd)[b, k] + b_attn[k]; all on PE+ACT
                # so no DVE-queue ordering hazard with the per-block bias adds.
                for h in range(NKH):
                    ps = spool.tile([BPC, KH], F32, tag="psetup")
                    for hb in range(HB):
                        nc.tensor.matmul(
                            ps[:], dect[:, hb, :], wd16[:, hb, h * KH : (h + 1) * KH],
                            start=(hb == 0), stop=False,
                        )
                    nc.tensor.matmul(
                        ps[:], ones1[:], brow[:, h * KH : (h + 1) * KH],
                        start=False, stop=True,
                    )
                    nc.scalar.activation(bias_all[:, h * KH : (h + 1) * KH], ps[:], AF.Copy)
                    nc.scalar.dma_start(
                        out=bias_dram[:, h, :], in_=bias_all[:, h * KH : (h + 1) * KH]
                    )

            def emit_corr_prep():
                # u = (We8/SW - We16) @ v ; wt = (We8/SW) @ v ; all on DVE, f32 accum.
                # m1 = fp8(u*SM), m2 = fp8(-wt*SM).
                upart = wsetup.tile([128, HB, NKH], F32, tag="upart")
                wtpart = wsetup.tile([128, HB, NKH], F32, tag="wtpart")
                usum = wsetup.tile([128, HB], F32, tag="usum")
                wtsum = wsetup.tile([128, HB], F32, tag="wtsum")
                diff = wsetup.tile([128, HB, NKH, KH], F16, tag="diff")
                for j in range(HB):
                    for h in range(NKH):
                        ksl = slice(h * KH, (h + 1) * KH)
                        nc.vector.scalar_tensor_tensor(
                            out=diff[:, j, h, :], in0=we8[:, h, j, :], scalar=1.0 / SW,
                            in1=we16[:, j, ksl], op0=ALU.mult, op1=ALU.subtract,
                        )
                        nc.vector.scalar_tensor_tensor(
                            out=uscr[:], in0=diff[:, j, h, :], scalar=0.0,
                            in1=vrep[:, ksl], op0=ALU.bypass, op1=ALU.mult,
                            accum_out=upart[:, j, h : h + 1],
                        )
                        nc.vector.scalar_tensor_tensor(
                            out=uscr[:], in0=we8[:, h, j, :], scalar=1.0 / SW,
                            in1=vrep[:, ksl], op0=ALU.mult, op1=ALU.mult,
                            accum_out=wtpart[:, j, h : h + 1],
                        )
                nc.vector.tensor_tensor(usum[:], upart[:, :, 0], upart[:, :, 1], ALU.add)
                nc.vector.tensor_tensor(wtsum[:], wtpart[:, :, 0], wtpart[:, :, 1], ALU.add)
                nc.scalar.activation(m1[:, :, 0], usum[:], AF.Copy, scale=SM)
                nc.scalar.activation(m2[:, :, 0], wtsum[:], AF.Copy, scale=-SM)

            # ---------------- per-batch loads ----------------
            enct_tiles, enclo_tiles, kc_tiles, bias_tiles = {}, {}, {}, {}

            def load_enct(b, queue):
                t = tpool.tile([128, HB, R], F8, tag="enct")
                queue.dma_start(out=t[:].rearrange("p hb r -> p (hb r)"), in_=enct_ext[b])
                enct_tiles[b] = t

            def load_enclo(b, queue):
                t = lpool.tile([128, HB, R], F8, tag="enclo")
                queue.dma_start(out=t[:].rearrange("p hb r -> p (hb r)"), in_=enclo_ext[b])
                enclo_tiles[b] = t

            def load_meta(b):
                kc = rpool.tile([128, RB], F32, tag="kc")
                nc.scalar.dma_start(out=kc[:], in_=kc_ext[b])
                kc_tiles[b] = kc

            def load_bias(b):
                bb = bpool.tile([128, NKH, KH], F32, tag="biasb")
                for h, q in ((0, nc.scalar), (1, nc.sync)):
                    q.dma_start(
                        out=bb[:, h, :],
                        in_=bias_dram[b : b + 1, h, :].broadcast_to([128, KH]),
                    )
                bias_tiles[b] = bb

            # ---------------- per-batch compute ----------------
            def emit_batch(b):
                enct = enct_tiles.pop(b)
                enclo = enclo_tiles.pop(b)
                kc = kc_tiles.pop(b)
                bb = bias_tiles.pop(b)
                att = rpool.tile([128, RB], F16, tag="att")
                for rb in range(RB):
                    rsl = slice(rb * 128, (rb + 1) * 128)
                    pk = psum_pool.tile([128, NKH, KH], F32, tag="pmm")
                    for h in range(NKH):
                        for j in range(NDR):
                            nc.tensor.matmul(
                                pk[:, h, :],
                                enct[:, 2 * j : 2 * j + 2, rsl],
                                we8[:, h, 2 * j : 2 * j + 2, :],
                                start=(j == 0), stop=(j == NDR - 1),
                                perf_mode=DR,
                            )
                    esum = epool.tile([128, NKH, KH], F32, tag="esum")
                    nc.vector.scalar_tensor_tensor(
                        out=esum[:].rearrange("p a k -> p (a k)"),
                        in0=pk[:].rearrange("p a k -> p (a k)"),
                        scalar=G2,
                        in1=bb[:].rearrange("p a k -> p (a k)"),
                        op0=ALU.mult, op1=ALU.add,
                    )
                    eng = engpool.tile([128, NKH, KH], F16, tag="energy")
                    nc.scalar.activation(
                        eng[:].rearrange("p a k -> p (a k)"),
                        esum[:].rearrange("p a k -> p (a k)"),
                        AF.Tanh,
                    )
                    # fused v_w dot: prod = eng * vrep (2x fp16), then reduce
                    prod = engpool.tile([128, NKH, KH], F16, tag="prod")
                    nc.vector.tensor_tensor(
                        prod[:].rearrange("p a k -> p (a k)"),
                        eng[:].rearrange("p a k -> p (a k)"),
                        vrep[:],
                        ALU.mult,
                    )
                    with nc.allow_low_precision("fp16 att logits, ~5e-4 rel"):
                        nc.vector.tensor_reduce(
                            att[:, rb : rb + 1],
                            prod[:].rearrange("p a k -> p (a k)"),
                            mybir.AxisListType.X,
                            ALU.add,
                        )
                # correction dots: corr_row[r] = enc8[:,r]@m1 + enclo[:,r]@m2
                corrrow = rpool.tile([1, R], F32, tag="corrrow")
                for g in range(NCG):
                    csl = slice(g * CW, (g + 1) * CW)
                    pc = cpsum_pool.tile([128, CW], F32, tag="pcorr")
                    for j in range(NDR):
                        nc.tensor.matmul(
                            pc[0:1, :],
                            m1[:, 2 * j : 2 * j + 2, 0:1],
                            enct[:, 2 * j : 2 * j + 2, csl],
                            start=(j == 0), stop=False,
                            perf_mode=DR,
                        )
                    for j in range(NDR):
                        nc.tensor.matmul(
                            pc[0:1, :],
                            m2[:, 2 * j : 2 * j + 2, 0:1],
                            enclo[:, 2 * j : 2 * j + 2, csl],
                            start=False, stop=(j == NDR - 1),
                            perf_mode=DR,
                        )
                    nc.scalar.activation(corrrow[0:1, csl], pc[0:1, :], AF.Copy)
                nc.gpsimd.dma_start(
                    out=corr_dram[b : b + 1].rearrange("one rb p -> one (rb p)"),
                    in_=corrrow[0:1, :],
                )
                corrsb = rpool.tile([128, RB], F32, tag="corrsb")
                nc.gpsimd.dma_start(
                    out=corrsb[:], in_=corr_dram[b].rearrange("rb p -> p rb")
                )
                # att -= CTANH * corr * GAM
                att2 = rpool.tile([128, RB], F32, tag="att2")
                nc.vector.scalar_tensor_tensor(
                    out=att2[:], in0=corrsb[:], scalar=-CTANH * GAM, in1=att[:],
                    op0=ALU.mult, op1=ALU.add,
                )
                # softmax over live rows (pads have kc=0)
                e = rpool.tile([128, RB], F32, tag="e")
                nc.scalar.activation(e[:], att2[:], AF.Exp)
                ec = rpool.tile([128, RB], F32, tag="ec")
                nc.vector.tensor_tensor(ec[:], e[:], kc[:], ALU.mult)
                zcol = rpool.tile([128, 1], F32, tag="zcol")
                nc.vector.tensor_reduce(zcol[:], ec[:], mybir.AxisListType.X, ALU.add)
                zall = rpool.tile([128, 1], F32, tag="zall")
                nc.gpsimd.partition_all_reduce(zall[:], zcol[:], 128, bass_isa.ReduceOp.add)
                zr = rpool.tile([128, 1], F32, tag="zr")
                nc.vector.reciprocal(zr[:], zall[:])
                probs = rpool.tile([128, RB], F32, tag="probs")
                nc.vector.tensor_scalar(probs[:], ec[:], zr[:], None, ALU.mult)
                nc.gpsimd.dma_start(out=out_ext[b], in_=probs[:])

            # ---------------- setup + steady-state loop ----------------
            setup_stack = tc.tile_pool(name="wsetup", bufs=1)
            wsetup = setup_stack.__enter__()
            wd16 = wsetup.tile([128, HB, H], F16, tag="wd")
            we16 = wsetup.tile([128, HB, H], F16, tag="we16")
            dect = wsetup.tile([128, HB, BPC], F16, tag="dect")
            brow = wsetup.tile([1, H], F16, tag="brow")
            ones1 = wsetup.tile([1, BPC], F16, tag="ones1")
            uscr = wsetup.tile([128, KH], F16, tag="uscr")
            nc.scalar.dma_start(out=dect[:].rearrange("p hb b -> p (hb b)"), in_=dect_ext[:])
            nc.scalar.dma_start(out=brow[:], in_=brow_ext[:])
            nc.scalar.dma_start(out=ones1[:], in_=ones_ext[:])
            nc.sync.dma_start(out=wd16[:].rearrange("p hb k -> p (hb k)"), in_=wd_ext[:])
            nc.sync.dma_start(
                out=we8[:].rearrange("p a hb k -> p (a hb k)"), in_=we8_ext[:]
            )
            nc.scalar.dma_start(
                out=we16[:].rearrange("p hb k -> p (hb k)"), in_=we16_ext[:]
            )
            load_meta(0)
            load_enct(0, nc.sync)
            load_enclo(0, nc.gpsimd)
            load_meta(1)
            emit_bias_setup()
            emit_corr_prep()
            load_bias(0)
            load_bias(1)

            for b in range(BPC):
                emit_batch(b)
                if b == 0:
                    load_enct(1, nc.sync)
                    load_enclo(1, nc.gpsimd)
                if b + 2 < BPC:
                    load_meta(b + 2)
                    load_enct(b + 2, nc.sync)
                    load_enclo(b + 2, nc.gpsimd)
                    load_bias(b + 2)
                if b == 0:
                    setup_stack.__exit__(None, None, None)

    nc.compile()
    return nc


def _get_graph(R=R_DEFAULT):
    if R not in _graph_cache:
        _graph_cache[R] = _build(R)
    return _graph_cache[R]


def _prep(enc, msk):
    """Host-side data movement: per-batch compaction + fp8 cast + transpose."""
    counts = (msk == 0).sum(axis=1)
    R = max(R_DEFAULT, int(-(-counts.max() // 128) * 128))
    RB = R // 128

    # fp8 split of enc: enc8 = q(enc*SE), enclo = q(enc*SE - enc8)
    encs = enc.astype(np.float32) * SE
    enc8f = np.clip(encs, -240, 240).astype(E4NP)
    enclof = (encs - enc8f.astype(np.float32)).astype(E4NP)

    encT = np.zeros((NCORES, BPC, 128, HB * R), E4NP)
    encL = np.zeros((NCORES, BPC, 128, HB * R), E4NP)
    kc = np.zeros((NCORES, BPC, 128, RB), np.float32)
    idxs = []
    for ci in range(NCORES):
        row = []
        for b in range(BPC):
            idx = np.where(msk[ci * BPC + b] == 0)[0]
            n = len(idx)
            for src, dst in ((enc8f, encT), (enclof, encL)):
                comp = np.zeros((R, H), E4NP)
                comp[:n] = src[ci * BPC + b, idx, :]
                # [R, H] -> [H, R] -> [HB, 128, R] -> [128, HB, R]
                t = comp.T.reshape(HB, 128, R).transpose(1, 0, 2)
                dst[ci, b] = t.reshape(128, HB * R)
            # row r = rb*128 + p lives at kc[p, rb]
            live = np.zeros(R, np.float32)
            live[:n] = 1.0
            kc[ci, b] = live.reshape(RB, 128).T
            row.append(idx)
        idxs.append(row)
    return R, encT, encL, kc, idxs


def _run(decoder_hidden, encoder_outputs, mask, W_attn, b_attn, v_w, **spmd_kwargs):
    from concourse.bass_utils import run_bass_kernel_spmd

    dec = np.asarray(decoder_hidden, dtype=np.float32)
    enc = np.asarray(encoder_outputs, dtype=np.float32)
    msk = np.asarray(mask, dtype=np.int32)
    W = np.asarray(W_attn, dtype=np.float32)
    bb = np.asarray(b_attn, dtype=np.float32)
    vv = np.asarray(v_w, dtype=np.float32)

    R, encT, encL, kc, idxs = _prep(enc, msk)
    nc = _get_graph(R)

    # weight/vector payloads in on-chip layouts (pure data movement / casts)
    we8 = (
        np.clip(W[H:] * SW, -240, 240).astype(E4NP)
        .reshape(HB, 128, NKH, KH).transpose(1, 2, 0, 3).reshape(128, -1)
    )
    we16 = W[H:].astype(np.float16).reshape(HB, 128, H).transpose(1, 0, 2).reshape(128, -1)
    wd16 = W[:H].astype(np.float16).reshape(HB, 128, H).transpose(1, 0, 2).reshape(128, -1)
    vrep = np.ascontiguousarray(np.broadcast_to(vv.astype(np.float16), (128, H)))
    brow = bb.astype(np.float16).reshape(1, H)
    ones1 = np.ones((1, BPC), np.float16)

    in_maps = []
    for i in range(NCORES):
        sl = slice(i * BPC, (i + 1) * BPC)
        dect = dec[sl].T.astype(np.float16).reshape(HB, 128, BPC).transpose(1, 0, 2).reshape(128, -1)
        in_maps.append(
            {
                "encT": encT[i],
                "encLo": encL[i],
                "we8": np.ascontiguousarray(we8),
                "we16": np.ascontiguousarray(we16),
                "wd": np.ascontiguousarray(wd16),
                "dect": np.ascontiguousarray(dect),
                "brow": brow,
                "ones1": ones1,
                "vrep": vrep,
                "kc": kc[i],
            }
        )
    res = run_bass_kernel_spmd(nc, in_maps, core_ids=list(range(NCORES)), **spmd_kwargs)
    out = np.zeros((B, S), np.float32)
    for ci in range(NCORES):
        for b in range(BPC):
            idx = idxs[ci][b]
            # out[b] is [128, RB]; row r = rb*128+p -> transpose then flatten
            flat = res.results[ci]["out"][b].T.reshape(-1)
            out[ci * BPC + b, idx] = flat[: len(idx)]
    return out, res


def kernel(decoder_hidden, encoder_outputs, mask, W_attn, b_attn, v_w):
    out, _ = _run(decoder_hidden, encoder_outputs, mask, W_attn, b_attn, v_w)
    return out


# revision 10
# speedup vs baseline: 1.1224x; 1.0156x over previous
"""Bahdanau-style attention kernel for Trainium2, 8 NeuronCores, data-parallel over
batch, with mask-sparsity: masked positions (mask==1) contribute exactly 0 to the
softmax, so their rows of encoder_outputs are never computed.

Reference computation, per (b, s):
    energy = tanh(dec @ Wd + enc @ We + b_attn)          # [B,S,H]
    att    = energy @ v_w                                 # [B,S]
    att    = where(mask==1, -1e10, att)
    out    = softmax(att, axis=1)

Full shapes: B=64, S=2048, H=1024. Each core takes 8 batches.

The big GEMM (enc @ We) runs in fp8 e4m3 with perf_mode=DoubleRow (2 contraction
elements per PE cell per cycle -> ~2x streaming rate vs fp16).  Raw fp8 noise
fails the 2e-2 gate, so a rank-1 statistically-linearized correction recovers it:

    att -= c_s * [enc@(dWe@v) + denc@(We@v)] ,  c_s = 1 - sum_k(v_k tanh_k)^2 / sum v^2

where dWe = We8/SW - We (weight quantization error, computed ON DEVICE from an
fp16 copy of We) and denc ~ -enc_lo/SE (the shipped fp8 residual of enc).  The
correction is two length-H dot products per row, evaluated on the PE as a
moving-role DoubleRow GEMM into a 1-partition PSUM row, then transposed back to
the rows-on-partitions layout via a DRAM bounce.  Measured end-to-end error:
~8.6e-3 (norm) / 1.6e-2 (max), deterministic for the fixed harness inputs.

Device pipeline, per batch (rows on PSUM partitions, kout on the free axis):
  - one contiguous DMA pulls encT8 [128, HB*R] fp8 into SBUF (double-buffered),
    plus the fp8 residual encLo on a second queue.
  - per 128-row block: psum[rows, kout] += DR-matmul(encT8 2-chunk, We8 2-chunk),
    4 DoubleRow chunks x 2 kout-halves of 512.
  - DVE adds bias row with the 1/(SE*SW) descale; ACT tanh -> fp16.
  - DVE v_w dot (scalar_tensor_tensor + row-accumulate) and a second pass
    accumulating (v*tanh)^2 for c_s.
  - corr GEMM: psum_row[1, 384] += m1.T@encT8_chunk + m2.T@encLo_chunk; ACT
    copies to an SBUF row, DRAM-bounced back as [128, RB].
  - softmax: ACT exp, DVE pad-mask multiply + reduce, gpsimd partition
    all-reduce, DVE reciprocal + scale, compact probabilities DMA out.
"""
import numpy as np
import ml_dtypes

B, S, H = 64, 2048, 1024
NCORES = 8
BPC = B // NCORES          # batches per core
HB = H // 128              # h blocks (contraction)
NDR = HB // 2              # DoubleRow chunk pairs
NKH = 2                    # kout halves (512 each, one PSUM bank per half)
KH = H // NKH
R_DEFAULT = 1152           # padded live rows per batch (multiple of 128)
MW = 16                    # m-vector pad width (DR needs 16B stride on the 2-axis)

SE = 8.0                   # enc fp8 scale
SW = 64.0                  # We fp8 scale
SM = 1024.0                # correction-vector fp8 scale
G2 = 1.0 / (SE * SW)       # psum -> pre descale
GAM = 1.0 / (SE * SM)      # corr psum -> logit descale
CTANH = 0.58               # E[tanh'] linearization constant for the correction

E4NP = ml_dtypes.float8_e4m3   # TRN FP8_EXP4-compatible (max 240)

_graph_cache = {}


def _build(R=R_DEFAULT):
    import concourse.bass as bass
    import concourse.bacc as bacc
    import concourse.tile as tile
    from concourse import mybir
    from concourse import bass_isa

    F32 = mybir.dt.float32
    F16 = mybir.dt.float16
    F8 = mybir.dt.float8e4
    AF = mybir.ActivationFunctionType
    ALU = mybir.AluOpType
    DR = mybir.MatmulPerfMode.DoubleRow
    RB = R // 128
    CW = 384 if R % 384 == 0 else 128   # corr row-chunk (<=512, divides R)
    NCG = R // CW

    nc = bacc.Bacc(trn_type="TRN2", target_bir_lowering=False)

    enct_ext = nc.declare_dram_parameter("encT", [BPC, 128, HB * R], F8, isOutput=False)
    enclo_ext = nc.declare_dram_parameter("encLo", [BPC, 128, HB * R], F8, isOutput=False)
    we8_ext = nc.declare_dram_parameter("we8", [128, NKH * HB * KH], F8, isOutput=False)
    we16_ext = nc.declare_dram_parameter("we16", [128, HB * H], F16, isOutput=False)
    wd_ext = nc.declare_dram_parameter("wd", [128, HB * H], F16, isOutput=False)
    dect_ext = nc.declare_dram_parameter("dect", [128, HB * BPC], F16, isOutput=False)
    brow_ext = nc.declare_dram_parameter("brow", [1, H], F16, isOutput=False)
    ones_ext = nc.declare_dram_parameter("ones1", [1, BPC], F16, isOutput=False)
    vrep9_ext = nc.declare_dram_parameter("vrep9", [128, RB * H], F16, isOutput=False)
    kc_ext = nc.declare_dram_parameter("kc", [BPC, 128, RB], F32, isOutput=False)
    out_ext = nc.declare_dram_parameter("out", [BPC, 128, RB], F32, isOutput=True)

    bias_dram = nc.dram_tensor("bias_dram", [BPC, NKH, KH], F16)
    corr_dram = nc.dram_tensor("corr_dram", [BPC, RB, 128], F32)

    with tile.TileContext(nc) as tc:
        with (
            tc.tile_pool(name="weights", bufs=1) as wpool,
            tc.tile_pool(name="enct", bufs=2) as tpool,
            tc.tile_pool(name="enclo", bufs=2) as lpool,
            tc.tile_pool(name="biasb", bufs=2) as bpool,
            tc.tile_pool(name="esum", bufs=2) as epool,
            tc.tile_pool(name="energy", bufs=2) as engpool,
            tc.tile_pool(name="prod9", bufs=1) as ppool,
            tc.tile_pool(name="rows", bufs=3) as rpool,
            tc.tile_pool(name="psum_mm", bufs=2, space="PSUM") as psum_pool,
            tc.tile_pool(name="psum_corr", bufs=2, space="PSUM") as cpsum_pool,
            tc.tile_pool(name="psum_setup", bufs=1, space="PSUM") as spool,
        ):
            # ---------------- persistent tiles ----------------
            we8 = wpool.tile([128, NKH, HB, KH], F8, tag="we8")
            vrep9 = wpool.tile([128, RB, NKH, KH], F16, tag="vrep9")
            bias_all = wpool.tile([BPC, H], F16, tag="bias_all")
            m1 = wpool.tile([128, HB, MW], F8, tag="m1")
            m2 = wpool.tile([128, HB, MW], F8, tag="m2")
            nc.scalar.dma_start(
                out=vrep9[:].rearrange("p rb a k -> p (rb a k)"), in_=vrep9_ext[:]
            )

            def emit_bias_setup():
                # bias_all[b, k] = (dec @ Wd)[b, k] + b_attn[k]; all on PE+ACT
                # so no DVE-queue ordering hazard with the per-block bias adds.
                for h in range(NKH):
                    ps = spool.tile([BPC, KH], F32, tag="psetup")
                    for hb in range(HB):
                        nc.tensor.matmul(
                            ps[:], dect[:, hb, :], wd16[:, hb, h * KH : (h + 1) * KH],
                            start=(hb == 0), stop=False,
                        )
                    nc.tensor.matmul(
                        ps[:], ones1[:], brow[:, h * KH : (h + 1) * KH],
                        start=False, stop=True,
                    )
                    nc.scalar.activation(bias_all[:, h * KH : (h + 1) * KH], ps[:], AF.Copy)
                    nc.scalar.dma_start(
                        out=bias_dram[:, h, :], in_=bias_all[:, h * KH : (h + 1) * KH]
                    )

            def emit_corr_prep():
                # wt = (We8/SW) @ v ; w16v = We16 @ v ; u = wt - w16v = dWe @ v.
                # m1 = fp8(u*SM), m2 = fp8(-wt*SM).  Batched DVE passes.
                scratch = wsetup.tile([128, HB, NKH, KH], F16, tag="scratch")
                wtsum = wsetup.tile([128, HB], F16, tag="wtsum")
                w16v = wsetup.tile([128, HB], F16, tag="w16v")
                usum = wsetup.tile([128, HB], F16, tag="usum")
                for h in range(NKH):
                    nc.vector.scalar_tensor_tensor(
                        out=scratch[:, :, h, :], in0=we8[:, h, :, :],
                        scalar=1.0 / SW, in1=vrep9[:, :HB, h, :],
                        op0=ALU.mult, op1=ALU.mult,
                    )
                with nc.allow_low_precision("fp16 corr vectors, 2nd-order"):
                    nc.vector.tensor_reduce(
                        wtsum[:], scratch[:], mybir.AxisListType.XY, ALU.add
                    )
                nc.vector.tensor_tensor(
                    scratch[:].rearrange("p hb a k -> p (hb a k)"),
                    we16[:].rearrange("p hb k -> p (hb k)"),
                    vrep9[:].rearrange("p rb a k -> p (rb a k)")[:, : HB * H],
                    ALU.mult,
                )
                with nc.allow_low_precision("fp16 corr vectors, 2nd-order"):
                    nc.vector.tensor_reduce(
                        w16v[:], scratch[:], mybir.AxisListType.XY, ALU.add
                    )
                nc.vector.tensor_tensor(usum[:], wtsum[:], w16v[:], ALU.subtract)
                nc.scalar.activation(m1[:, :, 0], usum[:], AF.Copy, scale=SM)
                nc.scalar.activation(m2[:, :, 0], wtsum[:], AF.Copy, scale=-SM)

            # ---------------- per-batch loads ----------------
            enct_tiles, enclo_tiles, kc_tiles, bias_tiles = {}, {}, {}, {}

            def load_enct(b, queue):
                t = tpool.tile([128, HB, R], F8, tag="enct")
                queue.dma_start(out=t[:].rearrange("p hb r -> p (hb r)"), in_=enct_ext[b])
                enct_tiles[b] = t

            def load_enclo(b, queue):
                t = lpool.tile([128, HB, R], F8, tag="enclo")
                queue.dma_start(out=t[:].rearrange("p hb r -> p (hb r)"), in_=enclo_ext[b])
                enclo_tiles[b] = t

            def load_meta(b):
                kc = rpool.tile([128, RB], F32, tag="kc")
                nc.scalar.dma_start(out=kc[:], in_=kc_ext[b])
                kc_tiles[b] = kc

            def load_bias(b):
                bb = bpool.tile([128, NKH, KH], F16, tag="biasb")
                for h, q in ((0, nc.scalar), (1, nc.sync)):
                    q.dma_start(
                        out=bb[:, h, :],
                        in_=bias_dram[b : b + 1, h, :].broadcast_to([128, KH]),
                    )
                bias_tiles[b] = bb

            # ---------------- per-batch compute ----------------
            soft_state = {}

            def emit_batch(b):
                enct = enct_tiles.pop(b)
                enclo = enclo_tiles.pop(b)
                bb = bias_tiles.pop(b)
                att = rpool.tile([128, RB], F16, tag="att")
                eng9 = engpool.tile([128, RB, NKH, KH], F16, tag="energy")
                for rb in range(RB):
                    rsl = slice(rb * 128, (rb + 1) * 128)
                    pk = psum_pool.tile([128, NKH, KH], F32, tag="pmm")
                    for h in range(NKH):
                        for j in range(NDR):
                            nc.tensor.matmul(
                                pk[:, h, :],
                                enct[:, 2 * j : 2 * j + 2, rsl],
                                we8[:, h, 2 * j : 2 * j + 2, :],
                                start=(j == 0), stop=(j == NDR - 1),
                                perf_mode=DR,
                            )
                    esum = epool.tile([128, NKH, KH], F16, tag="esum")
                    nc.vector.scalar_tensor_tensor(
                        out=esum[:].rearrange("p a k -> p (a k)"),
                        in0=pk[:].rearrange("p a k -> p (a k)"),
                        scalar=G2,
                        in1=bb[:].rearrange("p a k -> p (a k)"),
                        op0=ALU.mult, op1=ALU.add,
                    )
                    nc.scalar.activation(
                        eng9[:, rb].rearrange("p a k -> p (a k)"),
                        esum[:].rearrange("p a k -> p (a k)"),
                        AF.Tanh,
                    )
                # batched v_w dot over the whole batch: 2x fp16 DVE passes
                prod9 = ppool.tile([128, RB, NKH, KH], F16, tag="prod9")
                nc.vector.tensor_tensor(
                    prod9[:].rearrange("p rb a k -> p (rb a k)"),
                    eng9[:].rearrange("p rb a k -> p (rb a k)"),
                    vrep9[:].rearrange("p rb a k -> p (rb a k)"),
                    ALU.mult,
                )
                with nc.allow_low_precision("fp16 att logits, ~5e-4 rel"):
                    nc.vector.tensor_reduce(
                        att[:],
                        prod9[:].rearrange("p rb a k -> p rb (a k)"),
                        mybir.AxisListType.X,
                        ALU.add,
                    )
                # correction dots: corr_row[r] = enc8[:,r]@m1 + enclo[:,r]@m2
                corrrow = rpool.tile([1, R], F32, tag="corrrow")
                for g in range(NCG):
                    csl = slice(g * CW, (g + 1) * CW)
                    pc = cpsum_pool.tile([128, CW], F32, tag="pcorr")
                    for j in range(NDR):
                        nc.tensor.matmul(
                            pc[0:1, :],
                            m1[:, 2 * j : 2 * j + 2, 0:1],
                            enct[:, 2 * j : 2 * j + 2, csl],
                            start=(j == 0), stop=False,
                            perf_mode=DR,
                        )
                    for j in range(NDR):
                        nc.tensor.matmul(
                            pc[0:1, :],
                            m2[:, 2 * j : 2 * j + 2, 0:1],
                            enclo[:, 2 * j : 2 * j + 2, csl],
                            start=False, stop=(j == NDR - 1),
                            perf_mode=DR,
                        )
                    nc.scalar.activation(corrrow[0:1, csl], pc[0:1, :], AF.Copy)
                nc.gpsimd.dma_start(
                    out=corr_dram[b : b + 1].rearrange("one rb p -> one (rb p)"),
                    in_=corrrow[0:1, :],
                )
                corrsb = rpool.tile([128, RB], F32, tag="corrsb")
                nc.gpsimd.dma_start(
                    out=corrsb[:], in_=corr_dram[b].rearrange("rb p -> p rb")
                )
                soft_state[b] = (att, corrsb, kc_tiles.pop(b))

            def emit_soft(b):
                att, corrsb, kc = soft_state.pop(b)
                # att -= CTANH * corr * GAM
                att2 = rpool.tile([128, RB], F32, tag="att2")
                nc.vector.scalar_tensor_tensor(
                    out=att2[:], in0=corrsb[:], scalar=-CTANH * GAM, in1=att[:],
                    op0=ALU.mult, op1=ALU.add,
                )
                # softmax over live rows (pads have kc=0)
                e = rpool.tile([128, RB], F32, tag="e")
                nc.scalar.activation(e[:], att2[:], AF.Exp)
                ec = rpool.tile([128, RB], F32, tag="ec")
                nc.vector.tensor_tensor(ec[:], e[:], kc[:], ALU.mult)
                zcol = rpool.tile([128, 1], F32, tag="zcol")
                nc.vector.tensor_reduce(zcol[:], ec[:], mybir.AxisListType.X, ALU.add)
                zall = rpool.tile([128, 1], F32, tag="zall")
                nc.gpsimd.partition_all_reduce(zall[:], zcol[:], 128, bass_isa.ReduceOp.add)
                zr = rpool.tile([128, 1], F32, tag="zr")
                nc.vector.reciprocal(zr[:], zall[:])
                probs = rpool.tile([128, RB], F32, tag="probs")
                nc.vector.tensor_scalar(probs[:], ec[:], zr[:], None, ALU.mult)
                nc.gpsimd.dma_start(out=out_ext[b], in_=probs[:])

            # ---------------- setup + steady-state loop ----------------
            setup_stack = tc.tile_pool(name="wsetup", bufs=1)
            wsetup = setup_stack.__enter__()
            wd16 = wsetup.tile([128, HB, H], F16, tag="wd")
            we16 = wsetup.tile([128, HB, H], F16, tag="we16")
            dect = wsetup.tile([128, HB, BPC], F16, tag="dect")
            brow = wsetup.tile([1, H], F16, tag="brow")
            ones1 = wsetup.tile([1, BPC], F16, tag="ones1")
            nc.scalar.dma_start(out=dect[:].rearrange("p hb b -> p (hb b)"), in_=dect_ext[:])
            nc.scalar.dma_start(out=brow[:], in_=brow_ext[:])
            nc.scalar.dma_start(out=ones1[:], in_=ones_ext[:])
            nc.sync.dma_start(out=wd16[:].rearrange("p hb k -> p (hb k)"), in_=wd_ext[:])
            nc.sync.dma_start(
                out=we8[:].rearrange("p a hb k -> p (a hb k)"), in_=we8_ext[:]
            )
            nc.scalar.dma_start(
                out=we16[:].rearrange("p hb k -> p (hb k)"), in_=we16_ext[:]
            )
            load_meta(0)
            load_enct(0, nc.sync)
            load_enclo(0, nc.gpsimd)
            load_meta(1)
            emit_corr_prep()
            emit_bias_setup()
            load_bias(0)
            load_bias(1)

            for b in range(BPC):
                emit_batch(b)
                if b > 0:
                    emit_soft(b - 1)
                if b == 0:
                    load_enct(1, nc.sync)
                    load_enclo(1, nc.gpsimd)
                if b + 2 < BPC:
                    load_meta(b + 2)
                    load_enct(b + 2, nc.sync)
                    load_enclo(b + 2, nc.gpsimd)
                    load_bias(b + 2)
                if b == 0:
                    setup_stack.__exit__(None, None, None)
            emit_soft(BPC - 1)

    nc.compile()
    return nc


def _get_graph(R=R_DEFAULT):
    if R not in _graph_cache:
        _graph_cache[R] = _build(R)
    return _graph_cache[R]


def _prep(enc, msk):
    """Host-side data movement: per-batch compaction + fp8 cast + transpose."""
    counts = (msk == 0).sum(axis=1)
    R = max(R_DEFAULT, int(-(-counts.max() // 128) * 128))
    RB = R // 128

    # fp8 split of enc: enc8 = q(enc*SE), enclo = q(enc*SE - enc8)
    encs = enc.astype(np.float32) * SE
    enc8f = np.clip(encs, -240, 240).astype(E4NP)
    enclof = (encs - enc8f.astype(np.float32)).astype(E4NP)

    encT = np.zeros((NCORES, BPC, 128, HB * R), E4NP)
    encL = np.zeros((NCORES, BPC, 128, HB * R), E4NP)
    kc = np.zeros((NCORES, BPC, 128, RB), np.float32)
    idxs = []
    for ci in range(NCORES):
        row = []
        for b in range(BPC):
            idx = np.where(msk[ci * BPC + b] == 0)[0]
            n = len(idx)
            for src, dst in ((enc8f, encT), (enclof, encL)):
                comp = np.zeros((R, H), E4NP)
                comp[:n] = src[ci * BPC + b, idx, :]
                # [R, H] -> [H, R] -> [HB, 128, R] -> [128, HB, R]
                t = comp.T.reshape(HB, 128, R).transpose(1, 0, 2)
                dst[ci, b] = t.reshape(128, HB * R)
            # row r = rb*128 + p lives at kc[p, rb]
            live = np.zeros(R, np.float32)
            live[:n] = 1.0
            kc[ci, b] = live.reshape(RB, 128).T
            row.append(idx)
        idxs.append(row)
    return R, encT, encL, kc, idxs


def _run(decoder_hidden, encoder_outputs, mask, W_attn, b_attn, v_w, **spmd_kwargs):
    from concourse.bass_utils import run_bass_kernel_spmd

    dec = np.asarray(decoder_hidden, dtype=np.float32)
    enc = np.asarray(encoder_outputs, dtype=np.float32)
    msk = np.asarray(mask, dtype=np.int32)
    W = np.asarray(W_attn, dtype=np.float32)
    bb = np.asarray(b_attn, dtype=np.float32)
    vv = np.asarray(v_w, dtype=np.float32)

    R, encT, encL, kc, idxs = _prep(enc, msk)
    nc = _get_graph(R)

    # weight/vector payloads in on-chip layouts (pure data movement / casts)
    we8 = (
        np.clip(W[H:] * SW, -240, 240).astype(E4NP)
        .reshape(HB, 128, NKH, KH).transpose(1, 2, 0, 3).reshape(128, -1)
    )
    we16 = W[H:].astype(np.float16).reshape(HB, 128, H).transpose(1, 0, 2).reshape(128, -1)
    wd16 = W[:H].astype(np.float16).reshape(HB, 128, H).transpose(1, 0, 2).reshape(128, -1)
    RB = R // 128
    vrep9 = np.ascontiguousarray(
        np.broadcast_to(vv.astype(np.float16), (128, RB, H)).reshape(128, RB * H)
    )
    brow = bb.astype(np.float16).reshape(1, H)
    ones1 = np.ones((1, BPC), np.float16)

    in_maps = []
    for i in range(NCORES):
        sl = slice(i * BPC, (i + 1) * BPC)
        dect = dec[sl].T.astype(np.float16).reshape(HB, 128, BPC).transpose(1, 0, 2).reshape(128, -1)
        in_maps.append(
            {
                "encT": encT[i],
                "encLo": encL[i],
                "we8": np.ascontiguousarray(we8),
                "we16": np.ascontiguousarray(we16),
                "wd": np.ascontiguousarray(wd16),
                "dect": np.ascontiguousarray(dect),
                "brow": brow,
                "ones1": ones1,
                "vrep9": vrep9,
                "kc": kc[i],
            }
        )
    res = run_bass_kernel_spmd(nc, in_maps, core_ids=list(range(NCORES)), **spmd_kwargs)
    out = np.zeros((B, S), np.float32)
    for ci in range(NCORES):
        for b in range(BPC):
            idx = idxs[ci][b]
            # out[b] is [128, RB]; row r = rb*128+p -> transpose then flatten
            flat = res.results[ci]["out"][b].T.reshape(-1)
            out[ci * BPC + b, idx] = flat[: len(idx)]
    return out, res


def kernel(decoder_hidden, encoder_outputs, mask, W_attn, b_attn, v_w):
    out, _ = _run(decoder_hidden, encoder_outputs, mask, W_attn, b_attn, v_w)
    return out
